# revision 5
# baseline (speedup 1.0000x reference)
"""MixtralMoE kernel for 8 Trainium2 NeuronCores.

Strategy (expert-parallel, per sharding hint):
  - Host computes gate logits / top-2 routing / softmax combine weights
    (tiny: [8192,2048]@[2048,8]) and gathers each expert's tokens — this is
    the "all-to-all tokens by routing decision" placement step.
  - Each of the 8 cores owns one expert and runs a fused FFN
    y = (silu(x@w1T) * (x@w3T)) @ w2T, scaled by the per-token combine
    weight, over that expert's ~2048 routed tokens.
  - Host scatter-adds the two expert outputs per token back into the
    full [B,T,H,DH] output.

Device kernel: fp32 storage, float32r matmuls (full PE rate at N>=256).
Token blocks of <=512; per block, weights stream once (w1,w3,w2 ~100MB),
activations stay resident in SBUF.
"""

import numpy as np

B, T, H, DH = 4, 2048, 16, 128
D = H * DH          # 2048
F = 4096
E = 8
TOP_K = 2
N_TOKENS = B * T    # 8192
P = 128
ND = D // P         # 16
NF = F // P         # 32
NCORES = 8


def _plan_blocks(C):
    """Split C (multiple of 128) into blocks <=512, each a multiple of 128,
    preferring >=256 so fp32r matmuls run at full rate."""
    blocks = []
    rem = C
    while rem > 512:
        blocks.append(512)
        rem -= 512
    if rem == 128 and blocks:
        blocks[-1] = 384
        blocks.append(256)
    elif rem > 0:
        blocks.append(rem)
    return blocks


def _build_ffn(C, blocks):
    import concourse.bacc as bacc
    import concourse.mybir as mybir

    from concourse.tile import TileContext

    f32 = mybir.dt.float32
    fr = mybir.dt.float32r
    AF = mybir.ActivationFunctionType

    NT = C // P
    nc = bacc.Bacc(None, target_bir_lowering=False)

    xT = nc.dram_tensor("xT", [ND, P, C], fr, kind="ExternalInput")
    w1L = nc.dram_tensor("w1L", [NF, P, ND, P], fr, kind="ExternalInput")
    w3L = nc.dram_tensor("w3L", [NF, P, ND, P], fr, kind="ExternalInput")
    w2T = nc.dram_tensor("w2T", [NF, P, D], fr, kind="ExternalInput")
    weT = nc.dram_tensor("weT", [P, NT], f32, kind="ExternalInput")
    y = nc.dram_tensor("y", [C, D], f32, kind="ExternalOutput")

    with TileContext(nc) as tc:
        with (
            tc.tile_pool(name="xt", bufs=ND + 4) as p_xt,
            tc.tile_pool(name="w13", bufs=4) as p_w13,
            tc.tile_pool(name="w2", bufs=4) as p_w2,
            tc.tile_pool(name="hu", bufs=NF + 2) as p_hu,
            tc.tile_pool(name="tmp", bufs=3) as p_tmp,
            tc.tile_pool(name="ys", bufs=4) as p_ys,
            tc.tile_pool(name="cst", bufs=1) as p_cst,
            tc.tile_pool(name="pg", bufs=2, space="PSUM") as p_pg,
            tc.tile_pool(name="pu", bufs=2, space="PSUM") as p_pu,
            tc.tile_pool(name="py", bufs=4, space="PSUM") as p_py,
        ):
            wet = p_cst.tile([P, NT], f32)
            nc.sync.dma_start(wet[:], weT[:])

            off = 0
            for TB in blocks:
                # load this block's activations, transposed: 16 x [128, TB]
                xts = []
                for d in range(ND):
                    t = p_xt.tile([P, TB], fr, tag="xt")
                    nc.sync.dma_start(t[:], xT[d, :, off:off + TB])
                    xts.append(t)

                # ---- layer 1: hT/uT tiles [128f, TB], contract over D ----
                hus = []
                for f in range(NF):
                    w1c = p_w13.tile([P, ND, P], fr, tag="w13")
                    nc.sync.dma_start(w1c[:], w1L[f])
                    w3c = p_w13.tile([P, ND, P], fr, tag="w13")
                    nc.sync.dma_start(w3c[:], w3L[f])
                    pg = p_pg.tile([P, TB], f32)
                    pu = p_pu.tile([P, TB], f32)
                    for d in range(ND):
                        nc.tensor.matmul(
                            pg[:], w1c[:, d, :],
                            xts[d][:],
                            start=(d == 0), stop=(d == ND - 1),
                        )
                    for d in range(ND):
                        nc.tensor.matmul(
                            pu[:], w3c[:, d, :],
                            xts[d][:],
                            start=(d == 0), stop=(d == ND - 1),
                        )
                    sil = p_tmp.tile([P, TB], f32, tag="tmp")
                    nc.scalar.activation(sil[:], pg[:], AF.Silu)
                    hu = p_hu.tile([P, TB], fr, tag="hu")
                    nc.vector.tensor_mul(hu[:], sil[:], pu[:])
                    hus.append(hu)

                # ---- layer 2: y tiles [128tok, 512d], contract over F ----
                ntsub = TB // P
                for dd in range(D // 512):
                    pys = [p_py.tile([P, 512], f32, tag="py", name=f"py{ts}")
                           for ts in range(ntsub)]
                    for f in range(NF):
                        w2c = p_w2.tile([P, 512], fr, tag="w2")
                        nc.sync.dma_start(
                            w2c[:], w2T[f, :, dd * 512:(dd + 1) * 512])
                        for ts in range(ntsub):
                            nc.tensor.matmul(
                                pys[ts][:],
                                hus[f][:, ts * P:(ts + 1) * P],
                                w2c[:],
                                start=(f == 0), stop=(f == NF - 1),
                            )
                    for ts in range(ntsub):
                        ysb = p_ys.tile([P, 512], f32, tag="ys")
                        ti = off // P + ts
                        nc.vector.tensor_scalar_mul(
                            ysb[:], pys[ts][:], wet[:, ti:ti + 1])
                        nc.sync.dma_start(
                            y[off + ts * P: off + (ts + 1) * P,
                              dd * 512:(dd + 1) * 512],
                            ysb[:])
                off += TB
    nc.finalize()
    return nc


def _route(x, gate_w):
    """Host routing: returns per-expert (token_ids, combine_weights)."""
    logits = x @ gate_w.T                                   # [N, E] fp32
    order = np.argsort(-logits, axis=1, kind="stable")
    top_idx = order[:, :TOP_K]                              # [N, 2]
    top_logit = np.take_along_axis(logits, top_idx, axis=1)
    m = top_logit.max(axis=1, keepdims=True)
    e = np.exp(top_logit - m)
    gw = (e / e.sum(axis=1, keepdims=True)).astype(np.float32)
    per_expert = []
    for ex in range(E):
        m0 = top_idx[:, 0] == ex
        m1 = top_idx[:, 1] == ex
        tok = np.nonzero(m0 | m1)[0]
        w = np.where(m0, gw[:, 0], 0.0) + np.where(m1, gw[:, 1], 0.0)
        per_expert.append((tok, w[tok].astype(np.float32)))
    return per_expert


_CACHE = {}


def kernel(stm, gate_w, w1, w2, w3):
    from concourse.bass_utils import run_bass_kernel_spmd

    stm = np.asarray(stm, dtype=np.float32)
    gate_w = np.asarray(gate_w, dtype=np.float32)
    w1 = np.asarray(w1, dtype=np.float32)
    w2 = np.asarray(w2, dtype=np.float32)
    w3 = np.asarray(w3, dtype=np.float32)

    x = stm.reshape(N_TOKENS, D)
    per_expert = _route(x, gate_w)

    maxc = max(len(tok) for tok, _ in per_expert)
    C = ((maxc + P - 1) // P) * P
    blocks = _plan_blocks(C)
    NT = C // P

    in_maps = []
    for ex in range(E):
        tok, w = per_expert[ex]
        cnt = len(tok)
        xg = np.zeros((C, D), dtype=np.float32)
        xg[:cnt] = x[tok]
        xTt = np.ascontiguousarray(xg.T).reshape(ND, P, C)
        w1L = np.ascontiguousarray(
            w1[ex].reshape(NF, P, ND, P).transpose(0, 3, 2, 1))
        w3L = np.ascontiguousarray(
            w3[ex].reshape(NF, P, ND, P).transpose(0, 3, 2, 1))
        w2Tt = np.ascontiguousarray(w2[ex].T).reshape(NF, P, D)
        wep = np.zeros(C, dtype=np.float32)
        wep[:cnt] = w
        weT = np.ascontiguousarray(wep.reshape(NT, P).T)
        in_maps.append(
            {"xT": xTt, "w1L": w1L, "w3L": w3L, "w2T": w2Tt, "weT": weT})

    key = (C, tuple(blocks))
    if key not in _CACHE:
        _CACHE[key] = _build_ffn(C, blocks)
    nc = _CACHE[key]

    res = run_bass_kernel_spmd(nc, in_maps, core_ids=list(range(NCORES)))

    out = np.zeros((N_TOKENS, D), dtype=np.float32)
    for ex in range(E):
        tok, _ = per_expert[ex]
        out[tok] += res.results[ex]["y"][:len(tok)]
    return out.reshape(B, T, H, DH)
